# revision 27
# baseline (speedup 1.0000x reference)
"""Causal self-attention Bass/Trainium2 kernel.

Problem: B=4, T=2048, D=768, NH=12 heads (dh=64), fp32 I/O.

Sharding (8 NeuronCores, no collectives):
  core = b * 2 + hg  for batch b in 0..3, head-group hg in 0..1.
  Each core computes 6 heads (hg*6 .. hg*6+5) of one batch:
    Q/K/V projections for its heads, causal attention, and the partial
    output projection y_part = Z_part @ Wo_part (row-split contraction).
  Host sums the two partial outputs per batch and adds bo.

Per-core kernel layout (everything transposed so the contraction dim is
on partitions; host pre-transposes, which is free):
  xT  [768, 2048]          QT/KT [384, 2048] (pairs of heads per 128-row tile)
  V'  [2048, 6*65]         (ones column appended per head -> softmax sums)
  S^T [128k, 512q] blocks, P = exp(S/8) (no max subtraction: |logits| < 10),
  O'  = V'.T @ P^T accumulated over k tiles -> row 64 holds softmax sums.
  Causal masking: QK/exp/PV all restricted to the valid column range per
  block (N-trim); the 128-wide diagonal strip is masked in-place on
  GpSimd via affine_select.
  Normalization: reciprocal of the sums row, broadcast across partitions
  with a K=1 ones-matmul on the PE (no DRAM round trips), multiply.
"""

import numpy as np
import ml_dtypes

import concourse.bass as bass
from concourse import bacc
import concourse.mybir as mybir
import concourse.tile as tile
from concourse.bass_utils import run_bass_kernel_spmd

B, T, D, NH, DH = 4, 2048, 768, 12, 64
HPC = 6          # heads per core
NPAIR = 3        # head pairs per core
TQ = 512         # query tile (free dim of S^T blocks)
NQT = T // TQ    # 4
TKB = 128        # key tile (partition dim of S^T blocks)
NKT = T // TKB   # 16
KD = D // 128    # 6 contraction tiles for the projections
VW = DH + 1      # 65: V plus ones column

_f32 = mybir.dt.float32
ST_DT = mybir.dt.bfloat16
NP_DT = ml_dtypes.bfloat16

# PV contraction in fp8e4 with DoubleRow (2 key-blocks per matmul).
# Measured: 1.4x on PV matmul time but rel err 2.8e-2 > the 2e-2 gate
# (e4m3 quantization of either P or V alone costs ~2e-2). Keep off.
FP8_PV = False
FP8 = mybir.dt.float8e4
VWP = 80         # padded per-(head,ko) V stride in fp8 mode (must be %16 B)
EXP_BIAS = -3.2  # exp(S/8 + EXP_BIAS): keeps P <= e^5.3 ~ 196 < 240 (TRN
                 # e4m3 max); the common factor cancels in the softmax
                 # normalization. max S/8 over the causal region is ~8.5.


def _build_program():
    nc = bacc.Bacc()
    xT_d = nc.dram_tensor("xT", [KD, 128, T], ST_DT, kind="ExternalInput")
    wq_d = nc.dram_tensor("wqT", [KD, 128, HPC * DH], ST_DT, kind="ExternalInput")
    wk_d = nc.dram_tensor("wkT", [KD, 128, HPC * DH], ST_DT, kind="ExternalInput")
    wv_d = nc.dram_tensor("wvT", [KD, 128, HPC * DH], ST_DT, kind="ExternalInput")
    wo_d = nc.dram_tensor("woT", [NPAIR, 128, D], ST_DT, kind="ExternalInput")
    bq_d = nc.dram_tensor("bqT", [128, NPAIR], _f32, kind="ExternalInput")
    bk_d = nc.dram_tensor("bkT", [128, NPAIR], _f32, kind="ExternalInput")
    bvb_d = nc.dram_tensor("bvb", [HPC * VW], _f32, kind="ExternalInput")
    yT_d = nc.dram_tensor("yT", [KD, 128, T], ST_DT, kind="ExternalOutput")

    with tile.TileContext(nc) as tc:
        with (
            tc.tile_pool(name="const", bufs=1) as const,
            tc.tile_pool(name="ptp", bufs=4) as ptp,
            tc.tile_pool(name="workp", bufs=8) as workp,
            tc.tile_pool(name="ps512", bufs=4, space="PSUM") as ps512,
            tc.tile_pool(name="ps1024", bufs=2, space="PSUM") as ps1024,
        ):
            # ---- constants / persistent tensors ----
            xT_sb = const.tile([128, KD, T], ST_DT)
            wq_sb = const.tile([128, KD, HPC * DH], ST_DT)
            wk_sb = const.tile([128, KD, HPC * DH], ST_DT)
            wv_sb = const.tile([128, KD, HPC * DH], ST_DT)
            wo_sb = const.tile([128, NPAIR, D], ST_DT)
            bq_sb = const.tile([128, NPAIR], _f32)
            bk_sb = const.tile([128, NPAIR], _f32)
            bvb_sb = const.tile([128, HPC * VW], _f32)
            qt_sb = const.tile([128, NPAIR, T], ST_DT)
            kt_sb = const.tile([128, NPAIR, T], ST_DT)
            if FP8_PV:
                # [keys, block-pair, head, block-in-pair, dh+ones(pad 80)]
                v_sb = const.tile([128, NKT // 2, HPC, 2, VWP], FP8)
            else:
                v_sb = const.tile([128, NKT, HPC * VW], ST_DT)
            zt_sb = const.tile([128, NPAIR, T], ST_DT)
            ones_sb = const.tile([128, 64], ST_DT)

            # preload the exp table on ACT while input DMAs stream
            actw = const.tile([1, 2], _f32)
            nc.vector.memset(actw, 0.0)
            nc.scalar.activation(
                out=actw, in_=actw,
                func=mybir.ActivationFunctionType.Exp, scale=1.0,
            )

            # ---- input DMAs: demand order, split into partition halves so
            # they spread across DMA queues ----
            def dma2(out, in_):
                nc.sync.dma_start(out=out[0:64], in_=in_[0:64])
                nc.sync.dma_start(out=out[64:128], in_=in_[64:128])

            dma2(wq_sb, wq_d[:, :, :].rearrange("k p c -> p k c"))
            dma2(wk_sb, wk_d[:, :, :].rearrange("k p c -> p k c"))
            sl0 = slice(0, TQ)
            dma2(xT_sb[:, :, sl0], xT_d[:, :, sl0].rearrange("k p c -> p k c"))
            dma2(wv_sb, wv_d[:, :, :].rearrange("k p c -> p k c"))
            nc.sync.dma_start(out=bq_sb, in_=bq_d[:, :])
            nc.sync.dma_start(out=bk_sb, in_=bk_d[:, :])
            for c in range(1, NQT):
                sl = slice(c * TQ, (c + 1) * TQ)
                dma2(xT_sb[:, :, sl], xT_d[:, :, sl].rearrange("k p c -> p k c"))
            dma2(wo_sb, wo_d[:, :, :].rearrange("k p c -> p k c"))
            bvb_ap = bvb_d[:]
            bvb_bcast = bass.AP(
                tensor=bvb_ap.tensor, offset=bvb_ap.offset,
                ap=[[0, 128]] + list(bvb_ap.ap),
            )
            nc.gpsimd.dma_start(out=bvb_sb, in_=bvb_bcast)

            nc.vector.memset(ones_sb, 1.0)
            ebias_sb = const.tile([128, 1], _f32)
            nc.vector.memset(ebias_sb, EXP_BIAS)
            # ones column of V' (softmax denominator accumulator)
            if FP8_PV:
                nc.vector.memset(v_sb[:, :, :, :, DH : DH + 1], 1.0)
            else:
                v_by_head = v_sb.rearrange("p m (h c) -> p m h c", c=VW)
                nc.vector.memset(v_by_head[:, :, :, DH:VW], 1.0)

            # PE warm-up: dense dummy matmuls with no DMA deps keep the
            # tensor engine busy through the input DMA ramp so the HAM
            # clock-gate is at 2.4 GHz when real work arrives.
            dummy_sb = const.tile([128, 128], ST_DT, name="dummy")
            nc.vector.memset(dummy_sb, 0.0)
            ps_warm = ps512.tile([128, 128], _f32, tag="ps512", name="ps_warm")
            NWARM = 80
            for i in range(NWARM):
                # one live accumulation group: independent start/stop matmuls
                # into the same tile get dead-code-eliminated down to one
                nc.tensor.matmul(
                    ps_warm, lhsT=dummy_sb, rhs=dummy_sb,
                    start=(i == 0), stop=(i == NWARM - 1),
                )
            nc.vector.tensor_copy(dummy_sb, ps_warm)

            # ---- interleaved projections / attention / out-projection ----
            # All projection and out-projection matmul groups are emitted as
            # unit closures through a filler queue so they execute inside the
            # (otherwise ACT-bound) attention loops: this fills PE idle slots
            # and keeps the HAM clock-gate at full speed.
            from collections import deque

            queue = deque()          # pending (key, closure) units
            pending = {}             # key -> remaining unit count in queue
            emitted = set()

            def qk_group_units(which, mt, nt):
                w_sb, b_sb, dest = (
                    (wq_sb, bq_sb, qt_sb) if which == "q" else (wk_sb, bk_sb, kt_sb)
                )
                state = {}
                units = []
                for kt in range(KD):
                    def u(kt=kt):
                        if kt == 0:
                            state["ps"] = ps512.tile(
                                [128, TQ], _f32, tag="ps512", name="psg")
                        nc.tensor.matmul(
                            state["ps"],
                            lhsT=w_sb[:, kt, mt * 128 : (mt + 1) * 128],
                            rhs=xT_sb[:, kt, nt * TQ : (nt + 1) * TQ],
                            start=(kt == 0), stop=(kt == KD - 1),
                        )
                    units.append(u)
                def fin():
                    nc.vector.tensor_tensor(
                        out=dest[:, mt, nt * TQ : (nt + 1) * TQ],
                        in0=state["ps"],
                        in1=b_sb[:, mt : mt + 1].to_broadcast((128, TQ)),
                        op=mybir.AluOpType.add,
                    )
                units.append(fin)
                return units

            def v_group_units(mt):
                # all three pairs at once: rhs N=384
                state = {}
                units = []
                for kt in range(KD):
                    def u(kt=kt):
                        if kt == 0:
                            state["ps"] = ps512.tile(
                                [128, HPC * DH], _f32, tag="ps512", name="psg")
                        nc.tensor.matmul(
                            state["ps"],
                            lhsT=xT_sb[:, kt, mt * 128 : (mt + 1) * 128],
                            rhs=wv_sb[:, kt, :],
                            start=(kt == 0), stop=(kt == KD - 1),
                        )
                    units.append(u)
                def fin():
                    if FP8_PV:
                        vdst = v_sb[:, mt // 2, :, mt % 2, 0:DH]
                    else:
                        vdst = v_by_head[:, mt, :, 0:DH]
                    nc.vector.tensor_tensor(
                        out=vdst,
                        in0=state["ps"].rearrange("p (h c) -> p h c", c=DH),
                        in1=bvb_sb.rearrange("p (h c) -> p h c", c=VW)[:, :, 0:DH],
                        op=mybir.AluOpType.add,
                    )
                units.append(fin)
                return units

            def o_group_units(mt, nt):
                state = {}
                units = []
                for kt in range(NPAIR):
                    def u(kt=kt):
                        if kt == 0:
                            state["ps"] = ps512.tile(
                                [128, TQ], _f32, tag="ps512", name="psg")
                        nc.tensor.matmul(
                            state["ps"],
                            lhsT=wo_sb[:, kt, mt * 128 : (mt + 1) * 128],
                            rhs=zt_sb[:, kt, nt * TQ : (nt + 1) * TQ],
                            start=(kt == 0), stop=(kt == NPAIR - 1),
                        )
                    units.append(u)
                def fin():
                    yt = workp.tile([128, TQ], ST_DT, tag="yt", name="yt")
                    nc.vector.tensor_copy(yt, state["ps"])
                    nc.sync.dma_start(
                        out=yT_d[mt, :, nt * TQ : (nt + 1) * TQ], in_=yt)
                units.append(fin)
                return units

            # the very last query tile's out-projection is the kernel's tail:
            # pre-accumulate the pair-0/1 contraction into SBUF while pair 2
            # is still computing, leaving only one matmul + add for the tail
            y01_sb = const.tile([128, KD, TQ], _f32)

            def o01_group_units(mt, nt):
                state = {}
                units = []
                for kt in range(2):
                    def u(kt=kt):
                        if kt == 0:
                            state["ps"] = ps512.tile(
                                [128, TQ], _f32, tag="ps512", name="psg")
                        nc.tensor.matmul(
                            state["ps"],
                            lhsT=wo_sb[:, kt, mt * 128 : (mt + 1) * 128],
                            rhs=zt_sb[:, kt, nt * TQ : (nt + 1) * TQ],
                            start=(kt == 0), stop=(kt == 1),
                        )
                    units.append(u)
                def fin():
                    nc.vector.tensor_copy(y01_sb[:, mt, :], state["ps"])
                units.append(fin)
                return units

            def o2_group_units(mt, nt):
                state = {}
                units = []
                def u():
                    state["ps"] = ps512.tile(
                        [128, TQ], _f32, tag="ps512", name="psg")
                    nc.tensor.matmul(
                        state["ps"],
                        lhsT=wo_sb[:, 2, mt * 128 : (mt + 1) * 128],
                        rhs=zt_sb[:, 2, nt * TQ : (nt + 1) * TQ],
                        start=True, stop=True,
                    )
                units.append(u)
                def fin():
                    # the adds and output DMAs of the six o2 groups are the
                    # kernel's very tail: do them in halves across two DMA
                    # queues so transfer overlaps the remaining adds
                    for h in range(2):
                        hs = slice(h * (TQ // 2), (h + 1) * (TQ // 2))
                        yt = workp.tile([128, TQ // 2], ST_DT, tag="yt", name="yt")
                        nc.vector.tensor_tensor(
                            out=yt, in0=state["ps"][:, hs], in1=y01_sb[:, mt, hs],
                            op=mybir.AluOpType.add,
                        )
                        q0 = nt * TQ + h * (TQ // 2)
                        eng = nc.sync if (mt + h) % 2 == 0 else nc.gpsimd
                        eng.dma_start(
                            out=yT_d[mt, :, q0 : q0 + TQ // 2], in_=yt)
                units.append(fin)
                return units

            def units_for(key):
                kind = key[0]
                if kind == "q" or kind == "k":
                    return qk_group_units(kind, key[1], key[2])
                if kind == "v":
                    return v_group_units(key[1])
                if kind == "o01":
                    return o01_group_units(key[1], key[2])
                if kind == "o2":
                    return o2_group_units(key[1], key[2])
                return o_group_units(key[1], key[2])

            def push(key):
                if key in emitted:
                    return
                emitted.add(key)
                us = units_for(key)
                pending[key] = len(us)
                for u in us:
                    queue.append((key, u))

            def pop_unit():
                key, u = queue.popleft()
                u()
                pending[key] -= 1
                if pending[key] == 0:
                    del pending[key]

            def consume(n):
                for _ in range(n):
                    if queue:
                        pop_unit()

            def require(keys):
                # emit everything still queued for these groups right now
                for key in keys:
                    push(key)
                while any(pending.get(k, 0) > 0 for k in keys):
                    pop_unit()

            # queue pair-0 projections and all V in qt-demand order
            for nt in range(NQT):
                push(("k", 0, nt))
                push(("q", 0, nt))
                for mt in range(4 * nt, 4 * nt + 4):
                    push(("v", mt))

            # ---- attention per head pair ----
            for p in range(NPAIR):
                qA = qt_sb[0:64, p, :]
                qB = qt_sb[64:128, p, :]
                kA = kt_sb[0:64, p, :]
                kB = kt_sb[64:128, p, :]
                if p + 1 < NPAIR:  # queue next pair's Q/K projections
                    for nt in range(NQT):
                        push(("k", p + 1, nt))
                        push(("q", p + 1, nt))
                else:
                    # pairs 0/1 finished all query tiles: pre-accumulate
                    # their share of the final query tile's out-projection
                    for mt in range(KD):
                        push(("o01", mt, 0))
                # last pair runs big-to-small so the final dependency
                # chain (attention -> normalize -> out-projection -> DMA)
                # is the shortest one
                qt_order = [3, 2, 1, 0] if p == NPAIR - 1 else [0, 1, 2, 3]
                for qt in qt_order:
                    nk = 4 * (qt + 1)
                    require([("q", p, qt)])
                    oA = ps512.tile([128, TQ], _f32, tag="ps512", name="oA")
                    oB = ps512.tile([128, TQ], _f32, tag="ps512", name="oB")
                    qsl = slice(qt * TQ, (qt + 1) * TQ)
                    pts = [None] * nk

                    def off_of(kt, qt=qt):
                        return max(0, (kt - 4 * qt) * TKB)

                    def emit_qk(kt, qt=qt, qsl=qsl):
                        off = off_of(kt)
                        sab = ps1024.tile([128, 2, TQ], _f32, tag="sab", name="sab")
                        ksl = slice(kt * TKB, (kt + 1) * TKB)
                        vsl = slice(qsl.start + off, qsl.stop)
                        nc.tensor.matmul(
                            sab[:, 0, off:], lhsT=kA[:, ksl], rhs=qA[:, vsl],
                            start=True, stop=True,
                        )
                        nc.tensor.matmul(
                            sab[:, 1, off:], lhsT=kB[:, ksl], rhs=qB[:, vsl],
                            start=True, stop=True,
                        )
                        if FP8_PV:
                            # pt slot kt%2 of the block-pair tile
                            if kt % 2 == 0:
                                pts[kt // 2] = ptp.tile(
                                    [128, 2, 2, TQ], FP8, tag="pt", name="pt")
                            pt = pts[kt // 2]
                            nc.scalar.activation(
                                out=pt[:, :, kt % 2, off:], in_=sab[:, :, off:],
                                func=mybir.ActivationFunctionType.Exp,
                                scale=0.125, bias=ebias_sb[:, 0:1],
                            )
                            if kt >= 4 * qt:  # diagonal strip: causal triangle
                                if kt % 2 == 0:
                                    nc.gpsimd.affine_select(
                                        out=pt[:, :, 0, off : off + TKB],
                                        in_=pt[:, :, 0, off : off + TKB],
                                        compare_op=mybir.AluOpType.is_ge,
                                        fill=0.0, base=0,
                                        pattern=[[0, 2], [1, TKB]],
                                        channel_multiplier=-1,
                                    )
                                else:
                                    # the pair-wide DoubleRow matmul reads
                                    # this slot from off-128 on, but exp only
                                    # wrote [off:]: zero the gap strip
                                    nc.gpsimd.memset(
                                        pt[:, :, 1, off - TKB : off], 0.0)
                                    nc.gpsimd.affine_select(
                                        out=pt[:, :, 1, off : off + TKB],
                                        in_=pt[:, :, 1, off : off + TKB],
                                        compare_op=mybir.AluOpType.is_ge,
                                        fill=0.0, base=0,
                                        pattern=[[0, 2], [1, TKB]],
                                        channel_multiplier=-1,
                                    )
                        else:
                            pt = ptp.tile([128, 2, TQ], ST_DT, tag="pt", name="pt")
                            nc.scalar.activation(
                                out=pt[:, :, off:], in_=sab[:, :, off:],
                                func=mybir.ActivationFunctionType.Exp,
                                scale=0.125,
                            )
                            if kt >= 4 * qt:
                                nc.gpsimd.affine_select(
                                    out=pt[:, :, off : off + TKB],
                                    in_=pt[:, :, off : off + TKB],
                                    compare_op=mybir.AluOpType.is_ge,
                                    fill=0.0, base=0,
                                    pattern=[[0, 2], [1, TKB]],
                                    channel_multiplier=-1,
                                )
                            pts[kt] = pt

                    if FP8_PV:
                        njj = nk // 2

                        def emit_pv(jj, njj=njj, p=p):
                            off = off_of(2 * jj)
                            st, sp = (jj == 0), (jj == njj - 1)
                            pt = pts[jj]
                            nc.tensor.matmul(
                                oA[0:VW, off:],
                                lhsT=v_sb[:, jj, 2 * p, :, 0:VW],
                                rhs=pt[:, 0, :, off:],
                                start=st, stop=sp,
                                perf_mode=mybir.MatmulPerfMode.DoubleRow,
                            )
                            nc.tensor.matmul(
                                oB[0:VW, off:],
                                lhsT=v_sb[:, jj, 2 * p + 1, :, 0:VW],
                                rhs=pt[:, 1, :, off:],
                                start=st, stop=sp,
                                perf_mode=mybir.MatmulPerfMode.DoubleRow,
                            )
                            pts[jj] = None

                        # software pipeline: QK/exp one block-pair ahead of
                        # the PV consumer; fillers keep PE dense
                        for kt in range(nk):
                            require([("k", p, kt // 4)])
                            emit_qk(kt)
                            if kt % 2 == 1 and kt >= 3:
                                require([("v", kt - 3), ("v", kt - 2)])
                                emit_pv((kt - 3) // 2)
                            consume(3)
                        require([("v", nk - 2), ("v", nk - 1)])
                        consume(2)
                        emit_pv(njj - 1)
                    else:
                        def emit_pv(kt, nk=nk, p=p):
                            off = off_of(kt)
                            st, sp = (kt == 0), (kt == nk - 1)
                            pt = pts[kt]
                            nc.tensor.matmul(
                                oA[0:VW, off:],
                                lhsT=v_sb[:, kt, (2 * p) * VW : (2 * p + 1) * VW],
                                rhs=pt[:, 0, off:],
                                start=st, stop=sp,
                            )
                            nc.tensor.matmul(
                                oB[0:VW, off:],
                                lhsT=v_sb[:, kt, (2 * p + 1) * VW : (2 * p + 2) * VW],
                                rhs=pt[:, 1, off:],
                                start=st, stop=sp,
                            )
                            pts[kt] = None

                        # fillers go BETWEEN this block's QK and the lagged
                        # PV so the PE has work while the exp->mask chain of
                        # the consumed block finishes
                        for kt in range(nk):
                            require([("k", p, kt // 4)])
                            emit_qk(kt)
                            consume(2)
                            if kt >= 3:
                                require([("v", kt - 3)])
                                emit_pv(kt - 3)
                            consume(1)
                        for r in range(nk - 3, nk):
                            require([("v", r)])
                            consume(1)
                            emit_pv(r)

                    # stage O' to SBUF in bf16 immediately (frees both PSUM
                    # banks; z is stored in bf16 downstream anyway, so the
                    # cast costs no extra accuracy)
                    oAc = workp.tile([65, TQ], ST_DT, tag="oAc", name="oAc")
                    oBc = workp.tile([65, TQ], ST_DT, tag="oBc", name="oBc")
                    nc.vector.tensor_copy(oAc, oA[0:VW, :])
                    nc.vector.tensor_copy(oBc, oB[0:VW, :])
                    consume(2)
                    # normalize by the accumulated softmax sums (row 64):
                    # broadcast the sums row across partitions with a K=1
                    # ones-matmul on the PE (bf16: fp32 operands would make
                    # the PE run a 2-pass LOW_HIGH matmul), then take the
                    # reciprocal of the broadcast tile (lane-parallel; a
                    # [1,512] reciprocal would be serial on one DVE lane).
                    bcA = ps512.tile([64, TQ], _f32, tag="ps512", name="bcA")
                    nc.tensor.matmul(
                        bcA, lhsT=ones_sb[64:65, :], rhs=oAc[64:65, :],
                        start=True, stop=True,
                    )
                    rbA = workp.tile([64, TQ], _f32, tag="rbA", name="rbA")
                    nc.vector.reciprocal_approx_fast(out=rbA, in_=bcA)
                    nc.vector.tensor_mul(zt_sb[0:64, p, qsl], oAc[0:64, :], rbA)
                    consume(2)
                    bcB = ps512.tile([64, TQ], _f32, tag="ps512", name="bcB")
                    nc.tensor.matmul(
                        bcB, lhsT=ones_sb[64:65, :], rhs=oBc[64:65, :],
                        start=True, stop=True,
                    )
                    rbB = workp.tile([64, TQ], _f32, tag="rbB", name="rbB")
                    nc.vector.reciprocal_approx_fast(out=rbB, in_=bcB)
                    ztmp = workp.tile([64, TQ], ST_DT, tag="ztmp", name="ztmp")
                    nc.vector.tensor_mul(ztmp, oBc[0:64, :], rbB)
                    nc.gpsimd.dma_start(out=zt_sb[64:128, p, qsl], in_=ztmp)

                    if p == NPAIR - 1:
                        # queue this qt's out-projection columns; they are
                        # consumed as fillers during the NEXT qt so the
                        # normalize chain has time to finish
                        for mt in range(KD):
                            push(("o2", mt, 0) if qt == 0 else ("o", mt, qt))

            # drain the tail of the out-projection
            while queue:
                pop_unit()

    if not nc.is_finalized():
        nc.finalize()
    return nc


_CACHE = {}


def get_program():
    key = ("v3", FP8_PV)
    if key not in _CACHE:
        _CACHE[key] = _build_program()
    return _CACHE[key]


def make_in_maps(x, wq, bq, wk, bk, wv, bv, wo, bo):
    x, wq, bq, wk, bk, wv, bv, wo, bo = (
        np.asarray(a, dtype=np.float32) for a in (x, wq, bq, wk, bk, wv, bv, wo, bo)
    )
    in_maps = []
    for core in range(8):
        b, hg = core // 2, core % 2
        sl = slice(hg * HPC * DH, (hg + 1) * HPC * DH)
        xT = np.ascontiguousarray(x[b].T).astype(NP_DT).reshape(KD, 128, T)
        wqT = np.ascontiguousarray(wq[sl, :].T).astype(NP_DT).reshape(KD, 128, HPC * DH)
        wkT = np.ascontiguousarray(wk[sl, :].T).astype(NP_DT).reshape(KD, 128, HPC * DH)
        wvT = np.ascontiguousarray(wv[sl, :].T).astype(NP_DT).reshape(KD, 128, HPC * DH)
        woT = np.ascontiguousarray(wo[:, sl].T).astype(NP_DT).reshape(NPAIR, 128, D)
        bqT = np.ascontiguousarray(bq[sl].reshape(NPAIR, 128).T)
        bkT = np.ascontiguousarray(bk[sl].reshape(NPAIR, 128).T)
        bvb = np.zeros((HPC, VW), np.float32)
        bvb[:, :DH] = bv[sl].reshape(HPC, DH)
        bvb[:, DH] = 1.0
        in_maps.append(
            dict(xT=xT, wqT=wqT, wkT=wkT, wvT=wvT, woT=woT,
                 bqT=bqT, bkT=bkT, bvb=bvb.reshape(-1))
        )
    return in_maps


def assemble_output(results, bo):
    y = np.zeros((B, T, D), np.float32)
    for core in range(8):
        y[core // 2] += results[core]["yT"].astype(np.float32).reshape(D, T).T
    y += np.asarray(bo, np.float32)[None, None, :]
    return y


def kernel(**inputs):
    nc = get_program()
    in_maps = make_in_maps(**inputs)
    res = run_bass_kernel_spmd(nc, in_maps, core_ids=list(range(8)))
    return assemble_output(res.results, inputs["bo"])


if __name__ == "__main__":
    nc = get_program()
    print("program built OK")


# revision 31
# speedup vs baseline: 1.0178x; 1.0178x over previous
"""Causal self-attention Bass/Trainium2 kernel.

Problem: B=4, T=2048, D=768, NH=12 heads (dh=64), fp32 I/O.

Sharding (8 NeuronCores, no collectives):
  core = b * 2 + hg  for batch b in 0..3, head-group hg in 0..1.
  Each core computes 6 heads (hg*6 .. hg*6+5) of one batch:
    Q/K/V projections for its heads, causal attention, and the partial
    output projection y_part = Z_part @ Wo_part (row-split contraction).
  Host sums the two partial outputs per batch and adds bo.

Per-core kernel layout (everything transposed so the contraction dim is
on partitions; host pre-transposes, which is free):
  xT  [768, 2048]          QT/KT [384, 2048] (pairs of heads per 128-row tile)
  V'  [2048, 6*65]         (ones column appended per head -> softmax sums)
  S^T [128k, 512q] blocks, P = exp(S/8) (no max subtraction: |logits| < 10),
  O'  = V'.T @ P^T accumulated over k tiles -> row 64 holds softmax sums.
  Causal masking: QK/exp/PV all restricted to the valid column range per
  block (N-trim); the 128-wide diagonal strip is masked in-place on
  GpSimd via affine_select.
  Normalization: reciprocal of the sums row, broadcast across partitions
  with a K=1 ones-matmul on the PE (no DRAM round trips), multiply.
"""

import numpy as np
import ml_dtypes

import concourse.bass as bass
from concourse import bacc
import concourse.mybir as mybir
import concourse.tile as tile
from concourse.bass_utils import run_bass_kernel_spmd

B, T, D, NH, DH = 4, 2048, 768, 12, 64
HPC = 6          # heads per core
NPAIR = 3        # head pairs per core
TQ = 512         # query tile (free dim of S^T blocks)
NQT = T // TQ    # 4
TKB = 128        # key tile (partition dim of S^T blocks)
NKT = T // TKB   # 16
KD = D // 128    # 6 contraction tiles for the projections
VW = DH + 1      # 65: V plus ones column

_f32 = mybir.dt.float32
ST_DT = mybir.dt.bfloat16
NP_DT = ml_dtypes.bfloat16

# PV contraction in fp8e4 with DoubleRow (2 key-blocks per matmul).
# Measured: 1.4x on PV matmul time but rel err 2.8e-2 > the 2e-2 gate
# (e4m3 quantization of either P or V alone costs ~2e-2). Keep off.
FP8_PV = False
FP8 = mybir.dt.float8e4
VWP = 80         # padded per-(head,ko) V stride in fp8 mode (must be %16 B)
EXP_BIAS = -3.2  # exp(S/8 + EXP_BIAS): keeps P <= e^5.3 ~ 196 < 240 (TRN
                 # e4m3 max); the common factor cancels in the softmax
                 # normalization. max S/8 over the causal region is ~8.5.


def _build_program():
    nc = bacc.Bacc()
    xT_d = nc.dram_tensor("xT", [KD, 128, T], ST_DT, kind="ExternalInput")
    wq_d = nc.dram_tensor("wqT", [KD, 128, HPC * DH], ST_DT, kind="ExternalInput")
    wk_d = nc.dram_tensor("wkT", [KD, 128, HPC * DH], ST_DT, kind="ExternalInput")
    wv_d = nc.dram_tensor("wvT", [KD, 128, HPC * DH], ST_DT, kind="ExternalInput")
    wo_d = nc.dram_tensor("woT", [NPAIR, 128, D], ST_DT, kind="ExternalInput")
    bq_d = nc.dram_tensor("bqT", [128, NPAIR], _f32, kind="ExternalInput")
    bk_d = nc.dram_tensor("bkT", [128, NPAIR], _f32, kind="ExternalInput")
    bvb_d = nc.dram_tensor("bvb", [HPC * VW], _f32, kind="ExternalInput")
    yT_d = nc.dram_tensor("yT", [KD, 128, T], ST_DT, kind="ExternalOutput")

    with tile.TileContext(nc) as tc:
        with (
            tc.tile_pool(name="const", bufs=1) as const,
            tc.tile_pool(name="ptp", bufs=4) as ptp,
            tc.tile_pool(name="workp", bufs=8) as workp,
            tc.tile_pool(name="ps512", bufs=4, space="PSUM") as ps512,
            tc.tile_pool(name="ps1024", bufs=2, space="PSUM") as ps1024,
        ):
            # ---- constants / persistent tensors ----
            xT_sb = const.tile([128, KD, T], ST_DT)
            wq_sb = const.tile([128, KD, HPC * DH], ST_DT)
            wk_sb = const.tile([128, KD, HPC * DH], ST_DT)
            wv_sb = const.tile([128, KD, HPC * DH], ST_DT)
            wo_sb = const.tile([128, NPAIR, D], ST_DT)
            bq_sb = const.tile([128, NPAIR], _f32)
            bk_sb = const.tile([128, NPAIR], _f32)
            bvb_sb = const.tile([128, HPC * VW], _f32)
            qt_sb = const.tile([128, NPAIR, T], ST_DT)
            kt_sb = const.tile([128, NPAIR, T], ST_DT)
            if FP8_PV:
                # [keys, block-pair, head, block-in-pair, dh+ones(pad 80)]
                v_sb = const.tile([128, NKT // 2, HPC, 2, VWP], FP8)
            else:
                v_sb = const.tile([128, NKT, HPC * VW], ST_DT)
            zt_sb = const.tile([128, NPAIR, T], ST_DT)
            ones_sb = const.tile([128, 64], ST_DT)

            # preload the exp table on ACT while input DMAs stream
            actw = const.tile([1, 2], _f32)
            nc.vector.memset(actw, 0.0)
            nc.scalar.activation(
                out=actw, in_=actw,
                func=mybir.ActivationFunctionType.Exp, scale=1.0,
            )

            # ---- input DMAs: demand order, split into partition halves so
            # they spread across DMA queues ----
            def dma2(out, in_):
                nc.sync.dma_start(out=out[0:64], in_=in_[0:64])
                nc.sync.dma_start(out=out[64:128], in_=in_[64:128])

            dma2(wq_sb, wq_d[:, :, :].rearrange("k p c -> p k c"))
            dma2(wk_sb, wk_d[:, :, :].rearrange("k p c -> p k c"))
            sl0 = slice(0, TQ)
            dma2(xT_sb[:, :, sl0], xT_d[:, :, sl0].rearrange("k p c -> p k c"))
            dma2(wv_sb, wv_d[:, :, :].rearrange("k p c -> p k c"))
            nc.sync.dma_start(out=bq_sb, in_=bq_d[:, :])
            nc.sync.dma_start(out=bk_sb, in_=bk_d[:, :])
            for c in range(1, NQT):
                sl = slice(c * TQ, (c + 1) * TQ)
                dma2(xT_sb[:, :, sl], xT_d[:, :, sl].rearrange("k p c -> p k c"))
            dma2(wo_sb, wo_d[:, :, :].rearrange("k p c -> p k c"))
            bvb_ap = bvb_d[:]
            bvb_bcast = bass.AP(
                tensor=bvb_ap.tensor, offset=bvb_ap.offset,
                ap=[[0, 128]] + list(bvb_ap.ap),
            )
            nc.gpsimd.dma_start(out=bvb_sb, in_=bvb_bcast)

            nc.vector.memset(ones_sb, 1.0)
            ebias_sb = const.tile([128, 1], _f32)
            nc.vector.memset(ebias_sb, EXP_BIAS)
            # ones column of V' (softmax denominator accumulator)
            if FP8_PV:
                nc.vector.memset(v_sb[:, :, :, :, DH : DH + 1], 1.0)
            else:
                v_by_head = v_sb.rearrange("p m (h c) -> p m h c", c=VW)
                nc.vector.memset(v_by_head[:, :, :, DH:VW], 1.0)

            # PE warm-up: dense dummy matmuls with no DMA deps keep the
            # tensor engine busy through the input DMA ramp so the HAM
            # clock-gate is at 2.4 GHz when real work arrives.
            dummy_sb = const.tile([128, 128], ST_DT, name="dummy")
            nc.vector.memset(dummy_sb, 0.0)
            ps_warm = ps512.tile([128, 128], _f32, tag="ps512", name="ps_warm")
            NWARM = 140
            for i in range(NWARM):
                # one live accumulation group: independent start/stop matmuls
                # into the same tile get dead-code-eliminated down to one
                nc.tensor.matmul(
                    ps_warm, lhsT=dummy_sb, rhs=dummy_sb,
                    start=(i == 0), stop=(i == NWARM - 1),
                )
            nc.vector.tensor_copy(dummy_sb, ps_warm)

            # ---- interleaved projections / attention / out-projection ----
            # All projection and out-projection matmul groups are emitted as
            # unit closures through a filler queue so they execute inside the
            # (otherwise ACT-bound) attention loops: this fills PE idle slots
            # and keeps the HAM clock-gate at full speed.
            from collections import deque

            queue = deque()          # pending (key, closure) units
            pending = {}             # key -> remaining unit count in queue
            emitted = set()

            def qk_group_units(which, mt, nt):
                w_sb, b_sb, dest = (
                    (wq_sb, bq_sb, qt_sb) if which == "q" else (wk_sb, bk_sb, kt_sb)
                )
                state = {}
                units = []
                for kt in range(KD):
                    def u(kt=kt):
                        if kt == 0:
                            state["ps"] = ps512.tile(
                                [128, TQ], _f32, tag="ps512", name="psg")
                        nc.tensor.matmul(
                            state["ps"],
                            lhsT=w_sb[:, kt, mt * 128 : (mt + 1) * 128],
                            rhs=xT_sb[:, kt, nt * TQ : (nt + 1) * TQ],
                            start=(kt == 0), stop=(kt == KD - 1),
                        )
                    units.append(u)
                def fin():
                    nc.vector.tensor_tensor(
                        out=dest[:, mt, nt * TQ : (nt + 1) * TQ],
                        in0=state["ps"],
                        in1=b_sb[:, mt : mt + 1].to_broadcast((128, TQ)),
                        op=mybir.AluOpType.add,
                    )
                units.append(fin)
                return units

            def v_group_units(mt):
                # all three pairs at once: rhs N=384
                state = {}
                units = []
                for kt in range(KD):
                    def u(kt=kt):
                        if kt == 0:
                            state["ps"] = ps512.tile(
                                [128, HPC * DH], _f32, tag="ps512", name="psg")
                        nc.tensor.matmul(
                            state["ps"],
                            lhsT=xT_sb[:, kt, mt * 128 : (mt + 1) * 128],
                            rhs=wv_sb[:, kt, :],
                            start=(kt == 0), stop=(kt == KD - 1),
                        )
                    units.append(u)
                def fin():
                    if FP8_PV:
                        vdst = v_sb[:, mt // 2, :, mt % 2, 0:DH]
                    else:
                        vdst = v_by_head[:, mt, :, 0:DH]
                    nc.vector.tensor_tensor(
                        out=vdst,
                        in0=state["ps"].rearrange("p (h c) -> p h c", c=DH),
                        in1=bvb_sb.rearrange("p (h c) -> p h c", c=VW)[:, :, 0:DH],
                        op=mybir.AluOpType.add,
                    )
                units.append(fin)
                return units

            def o_group_units(mt, nt):
                state = {}
                units = []
                for kt in range(NPAIR):
                    def u(kt=kt):
                        if kt == 0:
                            state["ps"] = ps512.tile(
                                [128, TQ], _f32, tag="ps512", name="psg")
                        nc.tensor.matmul(
                            state["ps"],
                            lhsT=wo_sb[:, kt, mt * 128 : (mt + 1) * 128],
                            rhs=zt_sb[:, kt, nt * TQ : (nt + 1) * TQ],
                            start=(kt == 0), stop=(kt == NPAIR - 1),
                        )
                    units.append(u)
                def fin():
                    yt = workp.tile([128, TQ], ST_DT, tag="yt", name="yt")
                    nc.vector.tensor_copy(yt, state["ps"])
                    nc.sync.dma_start(
                        out=yT_d[mt, :, nt * TQ : (nt + 1) * TQ], in_=yt)
                units.append(fin)
                return units

            # the very last query tile's out-projection is the kernel's tail:
            # pre-accumulate the pair-0/1 contraction into SBUF while pair 2
            # is still computing, leaving only one matmul + add for the tail
            y01_sb = const.tile([128, KD, TQ], _f32)

            def o01_group_units(mt, nt):
                state = {}
                units = []
                for kt in range(2):
                    def u(kt=kt):
                        if kt == 0:
                            state["ps"] = ps512.tile(
                                [128, TQ], _f32, tag="ps512", name="psg")
                        nc.tensor.matmul(
                            state["ps"],
                            lhsT=wo_sb[:, kt, mt * 128 : (mt + 1) * 128],
                            rhs=zt_sb[:, kt, nt * TQ : (nt + 1) * TQ],
                            start=(kt == 0), stop=(kt == 1),
                        )
                    units.append(u)
                def fin():
                    nc.vector.tensor_copy(y01_sb[:, mt, :], state["ps"])
                units.append(fin)
                return units

            def o2_group_units(mt, nt):
                state = {}
                units = []
                def u():
                    state["ps"] = ps512.tile(
                        [128, TQ], _f32, tag="ps512", name="psg")
                    nc.tensor.matmul(
                        state["ps"],
                        lhsT=wo_sb[:, 2, mt * 128 : (mt + 1) * 128],
                        rhs=zt_sb[:, 2, nt * TQ : (nt + 1) * TQ],
                        start=True, stop=True,
                    )
                units.append(u)
                def fin():
                    # the adds and output DMAs of the six o2 groups are the
                    # kernel's very tail: do them in halves across two DMA
                    # queues so transfer overlaps the remaining adds
                    for h in range(2):
                        hs = slice(h * (TQ // 2), (h + 1) * (TQ // 2))
                        yt = workp.tile([128, TQ // 2], ST_DT, tag="yt", name="yt")
                        nc.vector.tensor_tensor(
                            out=yt, in0=state["ps"][:, hs], in1=y01_sb[:, mt, hs],
                            op=mybir.AluOpType.add,
                        )
                        q0 = nt * TQ + h * (TQ // 2)
                        eng = nc.sync if (mt + h) % 2 == 0 else nc.gpsimd
                        eng.dma_start(
                            out=yT_d[mt, :, q0 : q0 + TQ // 2], in_=yt)
                units.append(fin)
                return units

            def units_for(key):
                kind = key[0]
                if kind == "q" or kind == "k":
                    return qk_group_units(kind, key[1], key[2])
                if kind == "v":
                    return v_group_units(key[1])
                if kind == "o01":
                    return o01_group_units(key[1], key[2])
                if kind == "o2":
                    return o2_group_units(key[1], key[2])
                return o_group_units(key[1], key[2])

            def push(key):
                if key in emitted:
                    return
                emitted.add(key)
                us = units_for(key)
                pending[key] = len(us)
                for u in us:
                    queue.append((key, u))

            def pop_unit():
                key, u = queue.popleft()
                u()
                pending[key] -= 1
                if pending[key] == 0:
                    del pending[key]

            def consume(n):
                for _ in range(n):
                    if queue:
                        pop_unit()

            def require(keys):
                # emit everything still queued for these groups right now
                for key in keys:
                    push(key)
                while any(pending.get(k, 0) > 0 for k in keys):
                    pop_unit()

            # queue pair-0 projections and all V in qt-demand order
            for nt in range(NQT):
                push(("k", 0, nt))
                push(("q", 0, nt))
                for mt in range(4 * nt, 4 * nt + 4):
                    push(("v", mt))

            # ---- attention per head pair ----
            for p in range(NPAIR):
                qA = qt_sb[0:64, p, :]
                qB = qt_sb[64:128, p, :]
                kA = kt_sb[0:64, p, :]
                kB = kt_sb[64:128, p, :]
                if p + 1 < NPAIR:  # queue next pair's Q/K projections
                    for nt in range(NQT):
                        push(("k", p + 1, nt))
                        push(("q", p + 1, nt))

                # last pair runs big-to-small so the final dependency
                # chain (attention -> normalize -> out-projection -> DMA)
                # is the shortest one
                qt_order = [3, 2, 1, 0] if p == NPAIR - 1 else [0, 1, 2, 3]
                for qt in qt_order:
                    nk = 4 * (qt + 1)
                    require([("q", p, qt)])
                    oA = ps512.tile([128, TQ], _f32, tag="ps512", name="oA")
                    oB = ps512.tile([128, TQ], _f32, tag="ps512", name="oB")
                    qsl = slice(qt * TQ, (qt + 1) * TQ)
                    pts = [None] * nk

                    def off_of(kt, qt=qt):
                        return max(0, (kt - 4 * qt) * TKB)

                    def emit_qk(kt, qt=qt, qsl=qsl):
                        off = off_of(kt)
                        sab = ps1024.tile([128, 2, TQ], _f32, tag="sab", name="sab")
                        ksl = slice(kt * TKB, (kt + 1) * TKB)
                        vsl = slice(qsl.start + off, qsl.stop)
                        nc.tensor.matmul(
                            sab[:, 0, off:], lhsT=kA[:, ksl], rhs=qA[:, vsl],
                            start=True, stop=True,
                        )
                        nc.tensor.matmul(
                            sab[:, 1, off:], lhsT=kB[:, ksl], rhs=qB[:, vsl],
                            start=True, stop=True,
                        )
                        if FP8_PV:
                            # pt slot kt%2 of the block-pair tile
                            if kt % 2 == 0:
                                pts[kt // 2] = ptp.tile(
                                    [128, 2, 2, TQ], FP8, tag="pt", name="pt")
                            pt = pts[kt // 2]
                            nc.scalar.activation(
                                out=pt[:, :, kt % 2, off:], in_=sab[:, :, off:],
                                func=mybir.ActivationFunctionType.Exp,
                                scale=0.125, bias=ebias_sb[:, 0:1],
                            )
                            if kt >= 4 * qt:  # diagonal strip: causal triangle
                                if kt % 2 == 0:
                                    nc.gpsimd.affine_select(
                                        out=pt[:, :, 0, off : off + TKB],
                                        in_=pt[:, :, 0, off : off + TKB],
                                        compare_op=mybir.AluOpType.is_ge,
                                        fill=0.0, base=0,
                                        pattern=[[0, 2], [1, TKB]],
                                        channel_multiplier=-1,
                                    )
                                else:
                                    # the pair-wide DoubleRow matmul reads
                                    # this slot from off-128 on, but exp only
                                    # wrote [off:]: zero the gap strip
                                    nc.gpsimd.memset(
                                        pt[:, :, 1, off - TKB : off], 0.0)
                                    nc.gpsimd.affine_select(
                                        out=pt[:, :, 1, off : off + TKB],
                                        in_=pt[:, :, 1, off : off + TKB],
                                        compare_op=mybir.AluOpType.is_ge,
                                        fill=0.0, base=0,
                                        pattern=[[0, 2], [1, TKB]],
                                        channel_multiplier=-1,
                                    )
                        else:
                            pt = ptp.tile([128, 2, TQ], ST_DT, tag="pt", name="pt")
                            nc.scalar.activation(
                                out=pt[:, :, off:], in_=sab[:, :, off:],
                                func=mybir.ActivationFunctionType.Exp,
                                scale=0.125,
                            )
                            if kt >= 4 * qt:
                                nc.gpsimd.affine_select(
                                    out=pt[:, :, off : off + TKB],
                                    in_=pt[:, :, off : off + TKB],
                                    compare_op=mybir.AluOpType.is_ge,
                                    fill=0.0, base=0,
                                    pattern=[[0, 2], [1, TKB]],
                                    channel_multiplier=-1,
                                )
                            pts[kt] = pt

                    if FP8_PV:
                        njj = nk // 2

                        def emit_pv(jj, njj=njj, p=p):
                            off = off_of(2 * jj)
                            st, sp = (jj == 0), (jj == njj - 1)
                            pt = pts[jj]
                            nc.tensor.matmul(
                                oA[0:VW, off:],
                                lhsT=v_sb[:, jj, 2 * p, :, 0:VW],
                                rhs=pt[:, 0, :, off:],
                                start=st, stop=sp,
                                perf_mode=mybir.MatmulPerfMode.DoubleRow,
                            )
                            nc.tensor.matmul(
                                oB[0:VW, off:],
                                lhsT=v_sb[:, jj, 2 * p + 1, :, 0:VW],
                                rhs=pt[:, 1, :, off:],
                                start=st, stop=sp,
                                perf_mode=mybir.MatmulPerfMode.DoubleRow,
                            )
                            pts[jj] = None

                        # software pipeline: QK/exp one block-pair ahead of
                        # the PV consumer; fillers keep PE dense
                        for kt in range(nk):
                            require([("k", p, kt // 4)])
                            emit_qk(kt)
                            if kt % 2 == 1 and kt >= 3:
                                require([("v", kt - 3), ("v", kt - 2)])
                                emit_pv((kt - 3) // 2)
                            consume(3)
                        require([("v", nk - 2), ("v", nk - 1)])
                        consume(2)
                        emit_pv(njj - 1)
                    else:
                        def emit_pv(kt, nk=nk, p=p):
                            off = off_of(kt)
                            st, sp = (kt == 0), (kt == nk - 1)
                            pt = pts[kt]
                            nc.tensor.matmul(
                                oA[0:VW, off:],
                                lhsT=v_sb[:, kt, (2 * p) * VW : (2 * p + 1) * VW],
                                rhs=pt[:, 0, off:],
                                start=st, stop=sp,
                            )
                            nc.tensor.matmul(
                                oB[0:VW, off:],
                                lhsT=v_sb[:, kt, (2 * p + 1) * VW : (2 * p + 2) * VW],
                                rhs=pt[:, 1, off:],
                                start=st, stop=sp,
                            )
                            pts[kt] = None

                        # fillers go BETWEEN this block's QK and the lagged
                        # PV so the PE has work while the exp->mask chain of
                        # the consumed block finishes
                        for kt in range(nk):
                            require([("k", p, kt // 4)])
                            emit_qk(kt)
                            consume(2)
                            if kt >= 3:
                                require([("v", kt - 3)])
                                emit_pv(kt - 3)
                            consume(1)
                        for r in range(nk - 3, nk):
                            require([("v", r)])
                            consume(1)
                            emit_pv(r)

                    # stage O' to SBUF in bf16 immediately (frees both PSUM
                    # banks; z is stored in bf16 downstream anyway, so the
                    # cast costs no extra accuracy)
                    oAc = workp.tile([65, TQ], ST_DT, tag="oAc", name="oAc")
                    oBc = workp.tile([65, TQ], ST_DT, tag="oBc", name="oBc")
                    nc.vector.tensor_copy(oAc, oA[0:VW, :])
                    nc.vector.tensor_copy(oBc, oB[0:VW, :])
                    consume(2)
                    # normalize by the accumulated softmax sums (row 64):
                    # broadcast the sums row across partitions with a K=1
                    # ones-matmul on the PE (bf16: fp32 operands would make
                    # the PE run a 2-pass LOW_HIGH matmul), then take the
                    # reciprocal of the broadcast tile (lane-parallel; a
                    # [1,512] reciprocal would be serial on one DVE lane).
                    bcA = ps512.tile([64, TQ], _f32, tag="ps512", name="bcA")
                    nc.tensor.matmul(
                        bcA, lhsT=ones_sb[64:65, :], rhs=oAc[64:65, :],
                        start=True, stop=True,
                    )
                    rbA = workp.tile([64, TQ], _f32, tag="rbA", name="rbA")
                    nc.vector.reciprocal_approx_fast(out=rbA, in_=bcA)
                    nc.vector.tensor_mul(zt_sb[0:64, p, qsl], oAc[0:64, :], rbA)
                    consume(2)
                    bcB = ps512.tile([64, TQ], _f32, tag="ps512", name="bcB")
                    nc.tensor.matmul(
                        bcB, lhsT=ones_sb[64:65, :], rhs=oBc[64:65, :],
                        start=True, stop=True,
                    )
                    rbB = workp.tile([64, TQ], _f32, tag="rbB", name="rbB")
                    nc.vector.reciprocal_approx_fast(out=rbB, in_=bcB)
                    ztmp = workp.tile([64, TQ], ST_DT, tag="ztmp", name="ztmp")
                    nc.vector.tensor_mul(ztmp, oBc[0:64, :], rbB)
                    nc.sync.dma_start(out=zt_sb[64:128, p, qsl], in_=ztmp)

                    if p == NPAIR - 1:
                        # queue this qt's out-projection columns; they are
                        # consumed as fillers during the NEXT qt so the
                        # normalize chain has time to finish
                        for mt in range(KD):
                            push(("o2", mt, 0) if qt == 0 else ("o", mt, qt))
                        if qt == 1:
                            # pre-accumulate pairs 0/1's share of the final
                            # query tile's out-projection; queued HERE so
                            # these land as fillers right at the end, where
                            # the queue otherwise runs dry while the last
                            # normalize chain finishes
                            for mt in range(KD):
                                push(("o01", mt, 0))

            # drain the tail of the out-projection
            while queue:
                pop_unit()

    if not nc.is_finalized():
        nc.finalize()
    return nc


_CACHE = {}


def get_program():
    key = ("v3", FP8_PV)
    if key not in _CACHE:
        _CACHE[key] = _build_program()
    return _CACHE[key]


def make_in_maps(x, wq, bq, wk, bk, wv, bv, wo, bo):
    x, wq, bq, wk, bk, wv, bv, wo, bo = (
        np.asarray(a, dtype=np.float32) for a in (x, wq, bq, wk, bk, wv, bv, wo, bo)
    )
    in_maps = []
    for core in range(8):
        b, hg = core // 2, core % 2
        sl = slice(hg * HPC * DH, (hg + 1) * HPC * DH)
        xT = np.ascontiguousarray(x[b].T).astype(NP_DT).reshape(KD, 128, T)
        wqT = np.ascontiguousarray(wq[sl, :].T).astype(NP_DT).reshape(KD, 128, HPC * DH)
        wkT = np.ascontiguousarray(wk[sl, :].T).astype(NP_DT).reshape(KD, 128, HPC * DH)
        wvT = np.ascontiguousarray(wv[sl, :].T).astype(NP_DT).reshape(KD, 128, HPC * DH)
        woT = np.ascontiguousarray(wo[:, sl].T).astype(NP_DT).reshape(NPAIR, 128, D)
        bqT = np.ascontiguousarray(bq[sl].reshape(NPAIR, 128).T)
        bkT = np.ascontiguousarray(bk[sl].reshape(NPAIR, 128).T)
        bvb = np.zeros((HPC, VW), np.float32)
        bvb[:, :DH] = bv[sl].reshape(HPC, DH)
        bvb[:, DH] = 1.0
        in_maps.append(
            dict(xT=xT, wqT=wqT, wkT=wkT, wvT=wvT, woT=woT,
                 bqT=bqT, bkT=bkT, bvb=bvb.reshape(-1))
        )
    return in_maps


def assemble_output(results, bo):
    y = np.zeros((B, T, D), np.float32)
    for core in range(8):
        y[core // 2] += results[core]["yT"].astype(np.float32).reshape(D, T).T
    y += np.asarray(bo, np.float32)[None, None, :]
    return y


def kernel(**inputs):
    nc = get_program()
    in_maps = make_in_maps(**inputs)
    res = run_bass_kernel_spmd(nc, in_maps, core_ids=list(range(8)))
    return assemble_output(res.results, inputs["bo"])


if __name__ == "__main__":
    nc = get_program()
    print("program built OK")
